# revision 4
# baseline (speedup 1.0000x reference)
"""Local (sliding-window w=2) attention, B=4 S=2048 H=1024, on 8 trn2 cores.

Strategy: sequence-parallel. Each core owns half of one batch's sequence
(1024 tokens) plus a 2-token halo on each side (ext = 1028 tokens).

Q/K projections run in fp8(e4m3) with DoubleRow perf mode (2 contraction
rows packed per PE cell -> ~2x fp16 matmul throughput). x is scaled
by 32 and W by 2048 on the host; the PSUM result is unscaled + biased on
DVE during evacuation to fp16 Q^T/K^T. V projection and P@V stay fp16
(fp8 V fails the 2e-2 rel-err budget; fp8 Q/K lands at ~1.55e-2 measured
on the real inputs). Output is written fp16 and widened on the host.

DMA model (from perfetto): each dma_start binds to ONE SDMA queue and
moves one ~40-80ns line per SBUF partition; the scalar HWDGE ring owns
~13 queues, sync ~3. So the head of the stream is partition-split x4 so
the first Q matmul gates on ~1/4 of x8-th0 + 1/4 of wq chunk 0 (~10.4us
instead of 12.8us), and weight chunks are ordered in consumption order.
Output stores ride the scalar ring (13 queues ~= 0.8us/block) instead of
sync/gpsimd (3 queues ~= 3.3-5us/block, which serialized into a ~5us
post-compute tail).

K halo (ext tokens [1024,1028)) is folded into the main K t=1 pass as a
second 4-col DR matmul sharing each stationary -> ~0.8us instead of a
separate 64-matmul fp8 pass (~1.6us).

P@V is uniform 128-contraction: the P tile always carries 128 columns
(probs in [0,w), zeros in [w,127) for the short last block, ones at col
127), and V tiles are 3 persistent buffers whose row 127 holds the V
bias permanently (written once at startup), so no per-block bias-row DMA.

Per core: 9 q-blocks of 123 queries: band scores (window 127), masked
softmax (ACT exp + fused row-sum), P transpose on PE, P@V, fp16 out,
DMA. Blocks are software-pipelined one stage so softmax hides under the
next block's V projection.
"""

import os
import sys

sys.path.insert(0, "/opt/trn_rl_repo")

import ml_dtypes
import numpy as np

import concourse.bass as bass  # noqa: F401  (bass must import before tile)
import concourse.mybir as mybir
import concourse.tile as tile
from concourse import bacc
from concourse.bass_utils import run_bass_kernel_spmd

F32 = mybir.dt.float32
F16 = mybir.dt.float16
F8 = mybir.dt.float8e4
E4NP = ml_dtypes.float8_e4m3
DR = mybir.MatmulPerfMode.DoubleRow

B, S, H = 4, 2048, 1024
WCTX = 2
NCORES = 8
SHARD = S // 2  # tokens per core
EXT = SHARD + 2 * WCTX  # 1028
TH = 528  # fp8 x token-half width (514/516 used), 16B-aligned hc stride
P = 128
QB = 123  # queries per attention block
WIN = QB + 2 * WCTX  # 127 = key window per block
NBLK = (SHARD + QB - 1) // QB  # 9
HC = H // P  # 8 feature chunks
SCALE = 1.0 / np.sqrt(np.float32(H))
SX = 32.0  # host scale on x before fp8
SW = 2048.0  # host scale on W before fp8
UNSCALE = 1.0 / (SX * SW)

_prog_cache = {}


def _build_program():
    nc = bacc.Bacc("TRN2", target_bir_lowering=False, debug=False)
    # x8: token halves th0 = ext [0,514), th1 = ext [512,1028), each padded
    # to TH=528 cols so DoubleRow's hc stride is 16B-aligned and each DMA
    # half is one contiguous 4224B run per partition.
    x8_d = nc.dram_tensor("x8", [P, 2 * HC * TH], F8, kind="ExternalInput").ap()
    x16_d = nc.dram_tensor("x16", [P, HC * EXT], F16, kind="ExternalInput").ap()
    # wq/wk: [p, jh, jj, hc, 128] -> one contiguous 1KB run per partition
    # per (jh, jj) chunk; the DoubleRow hc-pair stride is 128B (16B-aligned).
    wq_d = nc.dram_tensor("wq", [P, 2 * 4 * HC * P], F8, kind="ExternalInput").ap()
    wk_d = nc.dram_tensor("wk", [P, 2 * 4 * HC * P], F8, kind="ExternalInput").ap()
    wv_d = nc.dram_tensor("wv", [P, HC * H], F16, kind="ExternalInput").ap()
    bq_d = nc.dram_tensor("bq_c", [P, HC], F32, kind="ExternalInput").ap()
    bk_d = nc.dram_tensor("bk_c", [P, HC], F32, kind="ExternalInput").ap()
    bv_d = nc.dram_tensor("bv1", [1, H], F16, kind="ExternalInput").ap()
    id_d = nc.dram_tensor("ident", [P, P], F16, kind="ExternalInput").ap()
    mk_d = nc.dram_tensor("mask", [QB, NBLK * WIN], F32, kind="ExternalInput").ap()
    out_d = nc.dram_tensor("out", [SHARD, H], F16, kind="ExternalOutput").ap()

    x8_r = x8_d.rearrange("p (th hc t) -> p th hc t", th=2, hc=HC)
    x16_r = x16_d.rearrange("p (hc t) -> p hc t", hc=HC)
    wq_r = wq_d.rearrange("p (jh jj hc j) -> p jh jj hc j", jh=2, jj=4, hc=HC)
    wk_r = wk_d.rearrange("p (jh jj hc j) -> p jh jj hc j", jh=2, jj=4, hc=HC)
    wv_r = wv_d.rearrange("p (hc j) -> p hc j", hc=HC)
    mk_r = mk_d.rearrange("q (b c) -> q b c", b=NBLK)

    with tile.TileContext(nc) as tc:
        with (
            tc.tile_pool(name="persist", bufs=1) as pers,
            tc.tile_pool(name="spool", bufs=2) as spool,
            tc.tile_pool(name="opool", bufs=4) as opool,
            tc.tile_pool(name="pproj", bufs=3, space="PSUM") as pproj,
            tc.tile_pool(name="patt", bufs=2, space="PSUM") as patt,
            tc.tile_pool(name="pout", bufs=2, space="PSUM") as pout,
            tc.tile_pool(name="ptp", bufs=1, space="PSUM") as ptp,
        ):
            # ---- gpsimd SWDGE (alive ~3us before the HWDGE rings): the
            # projection-evacuation biases ----
            bqc = pers.tile([P, HC], F32)
            nc.gpsimd.dma_start(bqc[:], bq_d)
            bkc = pers.tile([P, HC], F32)
            nc.gpsimd.dma_start(bkc[:], bk_d)
            x8_sb = pers.tile([P, 2, HC, TH], F8)
            wq_sb = pers.tile([P, 2, 4, HC, P], F8)
            wk_sb = pers.tile([P, 2, 4, HC, P], F8)
            x16_sb = pers.tile([P, HC, EXT], F16)
            wv_sb = pers.tile([P, HC, H], F16)
            maskt = pers.tile([QB, NBLK, WIN], F32)
            # ---- scalar ring: each dma_start binds one SDMA queue and its
            # per-partition lines serialize there, so the head of the stream
            # (x8 th0, wq chunks 0-4) is partition-split x4 to land on 4
            # parallel queues; the rest follows in consumption order. No PE
            # warmup: the first Q matmuls self-ramp the clock.
            PS = P // 4
            for s in range(4):
                nc.scalar.dma_start(
                    x8_sb[s * PS : (s + 1) * PS, 0], x8_r[s * PS : (s + 1) * PS, 0]
                )
            for s in range(4):
                nc.scalar.dma_start(
                    wq_sb[s * PS : (s + 1) * PS, 0, 0], wq_r[s * PS : (s + 1) * PS, 0, 0]
                )
            for jj in range(1, 4):
                for s in range(2):
                    h = P // 2
                    nc.scalar.dma_start(
                        wq_sb[s * h : (s + 1) * h, 0, jj],
                        wq_r[s * h : (s + 1) * h, 0, jj],
                    )
            for s in range(2):
                h = P // 2
                nc.scalar.dma_start(
                    wq_sb[s * h : (s + 1) * h, 1, 0], wq_r[s * h : (s + 1) * h, 1, 0]
                )
            for jj in range(1, 4):
                nc.scalar.dma_start(wq_sb[:, 1, jj], wq_r[:, 1, jj])
            for s in range(4):
                nc.scalar.dma_start(
                    x8_sb[s * PS : (s + 1) * PS, 1], x8_r[s * PS : (s + 1) * PS, 1]
                )
            for jh in range(2):
                for jj in range(4):
                    nc.scalar.dma_start(wk_sb[:, jh, jj], wk_r[:, jh, jj])
            nc.scalar.dma_start(x16_sb[:, :4], x16_r[:, :4])
            nc.scalar.dma_start(x16_sb[:, 4:], x16_r[:, 4:])
            nc.scalar.dma_start(wv_sb[:, :4], wv_r[:, :4])
            nc.scalar.dma_start(wv_sb[:, 4:], wv_r[:, 4:])
            nc.scalar.dma_start(maskt[:], mk_r)

            # ---- sync ring: attention-phase consts ----
            ident = pers.tile([P, P], F16)
            nc.sync.dma_start(ident[:], id_d)
            # V tiles: 3 persistent buffers; row 127 permanently holds the V
            # bias (the P tile's col 127 is the matching ones column), so
            # PV is always a uniform 128-row contraction.
            vbufs = [pers.tile([P, H], F16, name=f"vb{i}") for i in range(3)]
            for vb in vbufs:
                nc.sync.dma_start(vb[P - 1 : P, :], bv_d)

            # ---- Q^T projection: fp8 DoubleRow, owned tokens ext [2, 1026) ----
            # t-chunk 0 = th0 cols [2,514), t-chunk 1 = th1 cols [2,514)
            qt_sb = pers.tile([P, HC, SHARD], F16)
            for t in range(2):
                for jc in range(HC):
                    jh, jj = divmod(jc, 4)
                    ps = pproj.tile([P, 512], F32, tag="proj")
                    for c in range(4):
                        nc.tensor.matmul(
                            ps[:],
                            wq_sb[:, jh, jj, 2 * c : 2 * c + 2, :],
                            x8_sb[:, t, 2 * c : 2 * c + 2, 2:514],
                            start=(c == 0),
                            stop=(c == 3),
                            perf_mode=DR,
                        )
                    nc.vector.tensor_scalar(
                        qt_sb[:, jc, 512 * t : 512 * (t + 1)],
                        ps[:],
                        UNSCALE,
                        bqc[:, jc : jc + 1],
                        mybir.AluOpType.mult,
                        mybir.AluOpType.add,
                    )

            # ---- K^T projection: fp8 DR, ext tokens [0, 1024); the t=1 pass
            # also carries the halo [1024, 1028) as a second 4-col DR matmul
            # per stationary (th1 cols [512, 516)) into a small psum tile ----
            kt_sb = pers.tile([P, HC, EXT], F16)
            for t in range(2):
                for jc in range(HC):
                    jh, jj = divmod(jc, 4)
                    ps = pproj.tile([P, 512], F32, tag="proj")
                    psh = (
                        pproj.tile([P, 512], F32, tag="proj", name="psh")
                        if t == 1
                        else None
                    )
                    for c in range(4):
                        nc.tensor.matmul(
                            ps[:],
                            wk_sb[:, jh, jj, 2 * c : 2 * c + 2, :],
                            x8_sb[:, t, 2 * c : 2 * c + 2, 0:512],
                            start=(c == 0),
                            stop=(c == 3),
                            perf_mode=DR,
                        )
                        if t == 1:
                            nc.tensor.matmul(
                                psh[:, :4],
                                wk_sb[:, jh, jj, 2 * c : 2 * c + 2, :],
                                x8_sb[:, t, 2 * c : 2 * c + 2, 512:516],
                                start=(c == 0),
                                stop=(c == 3),
                                perf_mode=DR,
                            )
                    nc.vector.tensor_scalar(
                        kt_sb[:, jc, 512 * t : 512 * (t + 1)],
                        ps[:],
                        UNSCALE,
                        bkc[:, jc : jc + 1],
                        mybir.AluOpType.mult,
                        mybir.AluOpType.add,
                    )
                    if t == 1:
                        nc.vector.tensor_scalar(
                            kt_sb[:, jc, 1024:1028],
                            psh[:, :4],
                            UNSCALE,
                            bkc[:, jc : jc + 1],
                            mybir.AluOpType.mult,
                            mybir.AluOpType.add,
                        )

            # ---- attention blocks, software-pipelined by one stage ----
            def blk_geom(b):
                q0 = QB * b
                qb = min(QB, SHARD - q0)
                return q0, qb, qb + 2 * WCTX

            def emit_v(b):
                """V for block b's window, token-major [w, H], fp16, into the
                persistent buffer whose row 127 is the V bias."""
                q0, qb, w = blk_geom(b)
                vb = vbufs[b % 3]
                for n in range(2):
                    psv = pproj.tile([P, 512], F32, tag="proj")
                    for hc in range(HC):
                        nc.tensor.matmul(
                            psv[:w, :],
                            x16_sb[:, hc, q0 : q0 + w],
                            wv_sb[:, hc, 512 * n : 512 * (n + 1)],
                            start=(hc == 0),
                            stop=(hc == HC - 1),
                        )
                    nc.scalar.copy(vb[:w, 512 * n : 512 * (n + 1)], psv[:w, :])
                return vb

            def emit_scores_softmax(b):
                """Scores + masked softmax; returns normalized P tile (fp16)
                padded to 128 cols: probs [0,w), zeros [w,127), ones col 127."""
                q0, qb, w = blk_geom(b)
                pss = patt.tile([QB, WIN], F32, tag="ps")
                for jc in range(HC):
                    nc.tensor.matmul(
                        pss[:qb, :w],
                        qt_sb[:, jc, q0 : q0 + qb],
                        kt_sb[:, jc, q0 : q0 + w],
                        start=(jc == 0),
                        stop=(jc == HC - 1),
                    )
                sm = spool.tile([QB, WIN], F32, tag="sm")
                nc.vector.tensor_tensor(
                    sm[:qb, :w], pss[:qb, :w], maskt[:qb, b, :w], op=mybir.AluOpType.add
                )
                pexp = spool.tile([QB, WIN], F32, tag="pexp")
                rsum = spool.tile([QB, 1], F32, tag="rsum")
                nc.scalar.activation(
                    pexp[:qb, :w],
                    sm[:qb, :w],
                    mybir.ActivationFunctionType.Exp,
                    bias=0.0,
                    scale=float(SCALE),
                    accum_out=rsum[:qb],
                )
                rcp = spool.tile([QB, 1], F32, tag="rcp")
                nc.vector.reciprocal(rcp[:qb], rsum[:qb])
                pn = spool.tile([QB, P], F16, tag="pn")
                nc.vector.tensor_scalar_mul(pn[:qb, :w], pexp[:qb, :w], rcp[:qb])
                if w < WIN:
                    nc.vector.memset(pn[:qb, w : WIN], 0.0)
                nc.vector.memset(pn[:qb, WIN : P], 1.0)
                return pn

            def emit_ptranspose(b, pn):
                q0, qb, w = blk_geom(b)
                pst = ptp.tile([P, QB], F16, tag="pt")
                nc.tensor.transpose(pst[:, :qb], pn[:qb, :], ident[:qb, :qb])
                pts = spool.tile([P, QB], F16, tag="pts")
                nc.vector.tensor_copy(pts[:, :qb], pst[:, :qb])
                return pts

            def emit_pv_out(b, pn, vb, pts=None):
                """Transpose P (unless pre-hoisted), P@V, fp16 out, DMA."""
                q0, qb, w = blk_geom(b)
                if pts is None:
                    pts = emit_ptranspose(b, pn)
                ob = opool.tile([QB, H], F16, tag="ob")
                # two half-width PV psum tiles rotating in 2 bufs (same
                # 2-bank footprint as one full-width tile): the next PV
                # half only waits for the matching half's evacuation
                # instead of the whole previous block's
                for n in range(2):
                    pso = pout.tile([QB, 512], F32, tag="po")
                    nc.tensor.matmul(
                        pso[:qb, :],
                        pts[:, :qb],
                        vb[:, 512 * n : 512 * (n + 1)],
                        start=True,
                        stop=True,
                    )
                    eng = nc.vector.tensor_copy if n == 0 else nc.scalar.copy
                    eng(
                        ob[:qb, 512 * n : 512 * (n + 1)],
                        pso[:qb, :],
                    )
                # Stores: even blocks on the sync ring (3 queues, ~3.3us per
                # block at ~16us cadence), odd blocks + the last on the
                # scalar ring (13 queues, ~0.8us) -- the scalar engine only
                # stalls until ob is ready, and the input streams it shares
                # the ring with are fully landed by ~25us.
                eng = nc.sync if b % 2 == 0 and b != NBLK - 1 else nc.scalar
                eng.dma_start(out_d[q0 : q0 + qb, :], ob[:qb, :])

            # depth-2 pipeline: V and scores/softmax of blocks b+1, b+2 hide
            # under block b's transpose/PV on the PE
            # scores go first in each stage so the block's softmax chain
            # (DVE/ACT, ~2.3us) hides under the same stage's V projection
            # (3.4us of PE) — this matters most for the last blocks, where
            # no further V work exists to cover the chain.
            def emit_stage(b):
                pn = emit_scores_softmax(b)
                vb = emit_v(b)
                return (vb, pn)

            stage = []
            for b in range(min(2, NBLK)):
                stage.append(emit_stage(b))
            pts8 = None
            for b in range(NBLK):
                vb, pn = stage[b]
                emit_pv_out(b, pn, vb, pts8 if b == NBLK - 1 else None)
                if b + 2 < NBLK:
                    stage.append(emit_stage(b + 2))
                    if b + 2 == NBLK - 1:
                        # hoist the LAST block's transpose to right after
                        # its stage: the drain then runs PV-7, PV-8 back to
                        # back instead of threading a transpose->copy chain
                        # between them (safe only for block 8 — the pts/ptp
                        # ring rotations still see all consumers emitted
                        # before any buffer reuse)
                        pts8 = emit_ptranspose(NBLK - 1, stage[-1][1])

    nc.compile()
    return nc


def _build_mask(h: int) -> np.ndarray:
    mask = np.full((QB, NBLK, WIN), -1e30, dtype=np.float32)
    r = np.arange(QB)[:, None]
    c = np.arange(WIN)[None, :]
    band = (c - r >= 0) & (c - r <= 2 * WCTX)
    for b in range(NBLK):
        q0 = QB * b
        qb = min(QB, SHARD - q0)
        gk = h * SHARD + q0 + c - WCTX  # global key token index
        valid = band & (gk >= 0) & (gk < S) & (r < qb) & (c < qb + 2 * WCTX)
        mask[:, b, :] = np.where(valid, np.float32(0.0), np.float32(-1e30))
    return mask.reshape(QB, NBLK * WIN)


def _pack_rows(a: np.ndarray) -> np.ndarray:
    """[H, C] row-major -> [P, HC*C]: partition p line = rows p, 128+p, ..."""
    C = a.shape[1]
    return np.ascontiguousarray(
        a.reshape(HC, P, C).transpose(1, 0, 2).reshape(P, HC * C)
    )


def kernel(sequence_output, Wq, bq, Wk, bk, Wv, bv):
    x = np.asarray(sequence_output, dtype=np.float32)
    Wq = np.asarray(Wq, dtype=np.float32)
    Wk = np.asarray(Wk, dtype=np.float32)
    Wv = np.asarray(Wv, dtype=np.float32)
    bq = np.asarray(bq, dtype=np.float32)
    bk = np.asarray(bk, dtype=np.float32)
    bv = np.asarray(bv, dtype=np.float32)

    if "nc" not in _prog_cache:
        _prog_cache["nc"] = _build_program()
    nc = _prog_cache["nc"]

    def _pack_w8(W):
        """[H, H] -> [P, jh*jj*hc*128] with (p, jh, jj, hc, j) order, fp8."""
        a = (W * SW).astype(E4NP).view(np.uint8)  # rows h=hc*P+p, cols j
        a = a.reshape(HC, P, 2, 4, P).transpose(1, 2, 3, 0, 4)  # p,jh,jj,hc,j
        return np.ascontiguousarray(a.reshape(P, 2 * 4 * HC * P)).view(E4NP)

    wq8 = _pack_w8(Wq)
    wk8 = _pack_w8(Wk)
    wv_h = _pack_rows(Wv.astype(np.float16))
    bq_c = np.ascontiguousarray(bq.reshape(HC, P).T)
    bk_c = np.ascontiguousarray(bk.reshape(HC, P).T)
    bv_1 = np.ascontiguousarray(bv.reshape(1, H)).astype(np.float16)
    ident = np.eye(P, dtype=np.float16)
    masks = [_build_mask(0), _build_mask(1)]

    # pad each sequence with WCTX zero rows on both ends, slice ext windows
    xp = np.zeros((B, S + 2 * WCTX, H), dtype=np.float32)
    xp[:, WCTX : WCTX + S] = x

    in_maps = []
    for c in range(NCORES):
        bidx, h = divmod(c, 2)
        ext = xp[bidx, h * SHARD : h * SHARD + EXT]  # [EXT, H]
        xt = np.ascontiguousarray(ext.T)  # [H, EXT] f32
        x16 = _pack_rows(xt.astype(np.float16))
        # fp8 token halves: th0 = ext [0,514), th1 = ext [512,1028), pad TH=528
        x8q = (xt * SX).astype(E4NP)
        x8e = np.zeros((H, 2, TH), dtype=E4NP)
        x8e[:, 0, :514] = x8q[:, 0:514]
        x8e[:, 1, :516] = x8q[:, 512:1028]
        a = x8e.view(np.uint8).reshape(HC, P, 2, TH).transpose(1, 2, 0, 3)
        x8 = np.ascontiguousarray(a.reshape(P, 2 * HC * TH)).view(E4NP)
        in_maps.append(
            {
                "x8": x8,
                "x16": x16,
                "wq": wq8,
                "wk": wk8,
                "wv": wv_h,
                "bq_c": bq_c,
                "bk_c": bk_c,
                "bv1": bv_1,
                "ident": ident,
                "mask": masks[h],
            }
        )

    trace = bool(int(os.environ.get("LK_TRACE", "0")))
    res = run_bass_kernel_spmd(
        nc,
        in_maps,
        core_ids=list(range(NCORES)),
        trace=trace,
        trace_cores=list(range(NCORES)) if trace else None,
    )
    _prog_cache["last_results"] = res

    out = np.empty((B, S, H), dtype=np.float32)
    for c in range(NCORES):
        bidx, h = divmod(c, 2)
        out[bidx, h * SHARD : (h + 1) * SHARD] = res.results[c]["out"].astype(
            np.float32
        )
    return out
